# revision 56
# baseline (speedup 1.0000x reference)
"""Trainium2 Bass kernel for nn_CrossAttention (B=8, N=M=1024, D=1024, DK=768, H=16).

Sharding: data-parallel over batch B=8 -> one batch item per NeuronCore.
No collectives; attn.mean(dim=1) is over heads, all heads of a batch item
live on one core.

Per-core layout strategy:
  - Host pre-transposes activations and weights so every matmul has its
    contraction dim on SBUF partitions.
  - Scores are computed transposed: scoresT[m, n]. exp() is applied without
    max-subtraction (scores are bounded ~|2|; softmax value is unchanged).
  - QK^T for a head pair runs row-packed (64-row groups at tile_position
    (0,0)/(64,0)) so the two matmuls share the PE array concurrently.
  - PV uses a ones-augmented stationary [v_h | 1] so psum row 64 yields the
    softmax denominators for free.
  - The head-pair loop is software-pipelined: window p issues QK(p)
    interleaved with PV(p-1) at m-tile granularity to keep the PE stream
    dense; normalization/mean work of pair p-1 drains during window p.
  - attn_meanT = sum_h expT_h * (1/(16*sum_h[n])) accumulated on DVE in fp16;
    the host transposes the returned [m, n] array back to [n, m].
  - out = (outcatT * rbc16) @ (16*w_o.T) + b_o  (1/16 folded into the
    normalizer is compensated by scaling w_o.T by 16 on the host).
"""

import sys
import types

sys.path.insert(0, "/opt/trn_rl_repo")
sys.path.insert(0, "/root/.axon_site")

import numpy as np

N_CORES = 8
B, N, M = 8, 1024, 1024
D = 1024      # Q_DIM
DK = 768      # K_DIM
H = 16        # heads
HD = 64       # head dim
SCALE = HD ** -0.5  # 0.125


def _install_ntff_hook():
    """Make trace=True work under axon (antenv.axon_hooks shim)."""
    if "antenv.axon_hooks" in sys.modules:
        return
    try:
        import antenv
        hooks_mod = types.ModuleType("antenv.axon_hooks")
        _hook = [None]
        hooks_mod.set_axon_ntff_profile_hook = lambda h: _hook.__setitem__(0, h)
        hooks_mod.get_axon_ntff_profile_hook = lambda: _hook[0]
        sys.modules["antenv.axon_hooks"] = hooks_mod
        antenv.axon_hooks = hooks_mod
        from trn_agent_boot.trn_boot import _ntff_profile_via_ctypes
        hooks_mod.set_axon_ntff_profile_hook(
            _ntff_profile_via_ctypes("/opt/axon/libaxon_pjrt.so")
        )
    except Exception:
        pass


_CACHE = {}


def build_module():
    if "nc" in _CACHE:
        return _CACHE["nc"]

    import concourse.tile as tile
    import concourse.mybir as mybir
    from concourse import bacc, library_config

    f32 = mybir.dt.float32
    f32r = mybir.dt.float32r
    f16 = mybir.dt.float16
    AF = mybir.ActivationFunctionType

    nc = bacc.Bacc("TRN2", target_bir_lowering=False, debug=False,
                   num_devices=N_CORES)

    # ---- DRAM tensors (per-core shard) ----
    d_qT = nc.dram_tensor("qT_in", [D, N], f16, kind="ExternalInput").ap()
    d_kT = nc.dram_tensor("kT_in", [DK, M], f16, kind="ExternalInput").ap()
    d_vT = nc.dram_tensor("vT_in", [DK, M], f16, kind="ExternalInput").ap()
    d_wqT = nc.dram_tensor("wqT", [D, D], f16, kind="ExternalInput").ap()
    d_wkT = nc.dram_tensor("wkT", [DK, D], f16, kind="ExternalInput").ap()
    d_wvT = nc.dram_tensor("wvT", [DK, D], f16, kind="ExternalInput").ap()
    d_woT = nc.dram_tensor("woT16", [D, D], f16, kind="ExternalInput").ap()
    d_bq = nc.dram_tensor("bq_pp", [128, 8], f32, kind="ExternalInput").ap()
    d_bk = nc.dram_tensor("bk_pp", [128, 8], f32, kind="ExternalInput").ap()
    d_bv = nc.dram_tensor("bv_r", [1, D], f16, kind="ExternalInput").ap()
    d_bo = nc.dram_tensor("bo_r", [1, D], f16, kind="ExternalInput").ap()
    d_ones16 = nc.dram_tensor("ones16", [1, 512], f16, kind="ExternalInput").ap()
    d_out = nc.dram_tensor("out16", [N, D], f16, kind="ExternalOutput").ap()
    d_am16 = nc.dram_tensor("am16", [M, 2 * N], f16, kind="ExternalOutput").ap()

    with tile.TileContext(nc) as tc:
        nc.gpsimd.load_library(library_config.proxy)

        # ---------------- persistent pools ----------------
        const = tc.alloc_tile_pool(name="const", bufs=1)
        persist = tc.alloc_tile_pool(name="persist", bufs=1)

        ones16 = const.tile([1, 512], f16, tag="ones16", name="ones16")
        nc.sync.dma_start(ones16[:], d_ones16[:, :])
        bq_sb = const.tile([128, 8], f32, tag="bq", name="bq")
        bk_sb = const.tile([128, 8], f32, tag="bk", name="bk")

        qT = [persist.tile([128, N], f16, tag=f"qT{j}", name=f"qT{j}")
              for j in range(8)]
        kT = [persist.tile([128, M], f16, tag=f"kT{j}", name=f"kT{j}")
              for j in range(8)]
        v_sb = [persist.tile([128, H, HD + 1], f16, tag=f"v{j}", name=f"v{j}")
                for j in range(8)]
        outcat = [persist.tile([128, N], f16, tag=f"oc{j}", name=f"oc{j}")
                  for j in range(8)]
        # acc2[mj] = [sum_h normalized expT_A | sum_h ... B] (A cols 0:1024,
        # B cols 1024:2048); halves summed on host.
        acc2 = [persist.tile([128, 2 * N], f16, tag=f"acc{j}", name=f"acc{j}")
                for j in range(8)]

        for j in range(8):
            nc.vector.memset(v_sb[j][:, :, HD:HD + 1], 1.0)

        # ---------------- phase 1: projections ----------------
        wkv = tc.alloc_tile_pool(name="wkv", bufs=1)
        wk_t = [wkv.tile([128, D], f16, tag=f"wk{c}", name=f"wk{c}")
                for c in range(6)]
        wv_t = [wkv.tile([128, D], f16, tag=f"wv{c}", name=f"wv{c}")
                for c in range(6)]
        bv_sb = wkv.tile([1, D], f16, tag="bv", name="bv")

        wqp = tc.alloc_tile_pool(name="wqp", bufs=1)
        wq_t = [wqp.tile([128, D], f16, tag=f"wq{c}", name=f"wq{c}")
                for c in range(8)]

        # --- Q projection: qT[do, n] = wqT-chunks.T @ qT_in ---
        with tc.tile_pool(name="xq", bufs=4) as xp, \
             tc.tile_pool(name="proj_ps", bufs=1, space="PSUM") as pps:
            for nb in range(2):
                pss = [pps.tile([128, 512], f32, tag=f"ps{j}", name=f"ps{j}")
                       for j in range(8)]
                for c in range(8):
                    if nb == 0:
                        nc.sync.dma_start(wq_t[c][:],
                                          d_wqT[c * 128:(c + 1) * 128, :])
                    xt = xp.tile([128, 512], f16, tag="x", name="xt")
                    nc.sync.dma_start(
                        xt[:], d_qT[c * 128:(c + 1) * 128,
                                    nb * 512:(nb + 1) * 512])
                    for j in range(8):
                        nc.tensor.matmul(
                            pss[j][:],
                            wq_t[c][:, j * 128:(j + 1) * 128],
                            xt[:],
                            start=(c == 0), stop=(c == 7))
                if nb == 0:
                    nc.sync.dma_start(bq_sb[:], d_bq[:, :])
                    nc.sync.dma_start(bk_sb[:], d_bk[:, :])
                for j in range(8):
                    nc.scalar.activation(
                        qT[j][:, nb * 512:(nb + 1) * 512], pss[j][:],
                        AF.Identity, bias=bq_sb[:, j:j + 1], scale=1.0)
        wqp.release()


        with tc.tile_pool(name="xkv", bufs=4) as xp, \
             tc.tile_pool(name="proj_ps2", bufs=1, space="PSUM") as pps:
            # --- K projection ---
            for nb in range(2):
                pss = [pps.tile([128, 512], f32, tag=f"ps{j}", name=f"ps{j}")
                       for j in range(8)]
                for c in range(6):
                    if nb == 0:
                        nc.sync.dma_start(wk_t[c][:],
                                          d_wkT[c * 128:(c + 1) * 128, :])
                        nc.sync.dma_start(wv_t[c][:],
                                          d_wvT[c * 128:(c + 1) * 128, :])
                    xt = xp.tile([128, 1024], f16, tag="x", name="xt")
                    nc.sync.dma_start(
                        xt[:, 0:512], d_kT[c * 128:(c + 1) * 128,
                                           nb * 512:(nb + 1) * 512])
                    for j in range(8):
                        nc.tensor.matmul(
                            pss[j][:],
                            wk_t[c][:, j * 128:(j + 1) * 128],
                            xt[:, 0:512],
                            start=(c == 0), stop=(c == 5))
                if nb == 0:
                    nc.sync.dma_start(bv_sb[:], d_bv[:, :])
                for j in range(8):
                    nc.scalar.activation(
                        kT[j][:, nb * 512:(nb + 1) * 512], pss[j][:],
                        AF.Identity, bias=bk_sb[:, j:j + 1], scale=1.0)

            # --- V projection: v[m, do] = vT_in-chunks.T @ wvT (+ b_v) ---
            for ob in range(2):
                pss = [pps.tile([128, 512], f32, tag=f"ps{j}", name=f"ps{j}")
                       for j in range(8)]
                for c in range(6):
                    xt = xp.tile([128, 1024], f16, tag="x", name="xt")
                    nc.sync.dma_start(xt[:], d_vT[c * 128:(c + 1) * 128, :])
                    for mj in range(8):
                        nc.tensor.matmul(
                            pss[mj][:],
                            xt[:, mj * 128:(mj + 1) * 128],
                            wv_t[c][:, ob * 512:(ob + 1) * 512],
                            start=(c == 0), stop=False)
                for mj in range(8):
                    nc.tensor.matmul(
                        pss[mj][:],
                        ones16[:, 0:128],
                        bv_sb[:, ob * 512:(ob + 1) * 512],
                        start=False, stop=True)
                    nc.scalar.activation(
                        v_sb[mj][:, ob * 8:(ob + 1) * 8, 0:HD],
                        pss[mj][:].rearrange("p (a b) -> p a b", a=8),
                        AF.Copy)
        wkv.release()

        # w_o loaded during attention so O-projection starts without a stall
        wop = tc.alloc_tile_pool(name="wo", bufs=1)
        wo_t = [wop.tile([128, D], f16, tag=f"wo{c}", name=f"wo{c}")
                for c in range(8)]
        bo_sb = wop.tile([1, D], f16, tag="bo", name="bo")
        nc.sync.dma_start(bo_sb[:], d_bo[:, :])
        for c in range(8):
            nc.sync.dma_start(wo_t[c][:], d_woT[c * 128:(c + 1) * 128, :])

        # O-proj fp16 staging (bufs=1: copy/DMA alternate in the tail)
        osp = tc.alloc_tile_pool(name="ostage", bufs=1)

        # ---------------- phase 2: attention (software-pipelined pairs) ----
        with tc.tile_pool(name="exp", bufs=2) as expp, \
             tc.tile_pool(name="att_tmp", bufs=2) as tmpp, \
             tc.tile_pool(name="rinvp", bufs=1) as rinvp, \
             tc.tile_pool(name="rbc", bufs=1) as rbcp, \
             tc.tile_pool(name="qk_ps", bufs=1, space="PSUM") as qkps, \
             tc.tile_pool(name="pv_ps", bufs=1, space="PSUM") as pvps:

            exps = {}
            pvts = {}
            deferred = []
            for p in range(9):
                # ---- interleaved PE stream: QK(p) + PV(p-1) ----
                if p < 8:
                    cur = []
                    exps[p] = cur
                for mj in range(8):
                    if p < 8:
                        et = expp.tile([128, 2 * N], f16, tag=f"exp{mj}",
                                       name=f"exp{mj}", bufs=2)
                        psA = qkps.tile([128, 1024], f32, tag="qkA",
                                        name="qkA")
                        for nb in range(2):
                            nc.tensor.matmul(
                                psA[:, nb * 512:(nb + 1) * 512],
                                kT[p][0:64, mj * 128:(mj + 1) * 128],
                                qT[p][0:64, nb * 512:(nb + 1) * 512],
                                start=True, stop=True, tile_position=(0, 0))
                        nc.scalar.activation(et[:, 0:N], psA[:], AF.Exp,
                                             scale=SCALE)
                        psB = qkps.tile([128, 1024], f32, tag="qkB",
                                        name="qkB")
                        for nb in range(2):
                            nc.tensor.matmul(
                                psB[:, nb * 512:(nb + 1) * 512],
                                kT[p][64:128, mj * 128:(mj + 1) * 128],
                                qT[p][64:128, nb * 512:(nb + 1) * 512],
                                start=True, stop=True, tile_position=(64, 0))
                        nc.scalar.activation(et[:, N:2 * N], psB[:], AF.Exp,
                                             scale=SCALE)
                    if p >= 1:
                        pvA, pvB = pvts[p - 1]
                        c = exps[p - 1]
                        for nb in range(2):
                            nc.tensor.matmul(
                                pvA[:, nb * 512:(nb + 1) * 512],
                                v_sb[mj][:, 2 * (p - 1), :],
                                c[mj][:, nb * 512:(nb + 1) * 512],
                                start=(mj == 0), stop=(mj == 7))
                            nc.tensor.matmul(
                                pvB[:, nb * 512:(nb + 1) * 512],
                                v_sb[mj][:, 2 * (p - 1) + 1, :],
                                c[mj][:, N + nb * 512:N + (nb + 1) * 512],
                                start=(mj == 0), stop=(mj == 7))
                    if p < 8:
                        cur.append(et)
                if p < 8:
                    pvts[p] = (
                        pvps.tile([65, 1024], f32, tag="pvA", name="pvA"),
                        pvps.tile([65, 1024], f32, tag="pvB", name="pvB"))

                # ---- drain pair p-1 ----
                if p >= 1:
                    q = p - 1
                    pvA, pvB = pvts[q]
                    c = exps[q]
                    sCAT = rinvp.tile([1, 2048], f32, tag="sCAT",
                                      name="sCAT")
                    r16cat = rinvp.tile([1, 2048], f16, tag="r16cat",
                                        name="r16cat", bufs=1)
                    # den extraction first: it heads the critical chain
                    # den -> recip -> cast -> broadcast -> means
                    nc.scalar.copy(sCAT[:, 0:N], pvA[64:65, :])
                    nc.scalar.copy(sCAT[:, N:2 * N], pvB[64:65, :])
                    nc.vector.reciprocal_approx_fast(out=sCAT[:], in_=sCAT[:])
                    nc.vector.tensor_copy(r16cat[:], sCAT[:])
                    rbcAB = rbcp.tile([128, 2 * N], f16, tag="rbcAB",
                                      name="rbcAB", bufs=1)
                    nc.gpsimd.partition_broadcast(rbcAB[:], r16cat[:])

                    def mean_mj(mj, q=q, c=c, rbcAB=rbcAB):
                        # mj 6,7 on gpsimd; rest on DVE
                        eng = nc.gpsimd if mj >= 6 else nc.vector
                        tg = "atg" if mj >= 6 else "at"
                        if q == 0:
                            eng.tensor_mul(acc2[mj][:], c[mj][:], rbcAB[:])
                        else:
                            at = tmpp.tile([128, 2 * N], f16, tag=tg,
                                           name=tg, bufs=1)
                            eng.tensor_mul(at[:], c[mj][:], rbcAB[:])
                            eng.tensor_add(acc2[mj][:], acc2[mj][:], at[:])

                    def norms(q=q, rbcAB=rbcAB, pvA=pvA, pvB=pvB):
                        # fused psum-read normalize (copy+mul in one TT)
                        nc.vector.tensor_mul(outcat[q][0:64, :],
                                             pvA[0:64, :],
                                             rbcAB[0:64, 0:N])
                        nc.vector.tensor_mul(outcat[q][64:128, :],
                                             pvB[0:64, :],
                                             rbcAB[64:128, N:2 * N])

                    if p < 8:
                        # gpsimd means (7,6) queued right after its bcast;
                        # DVE: first head-norms, then its means
                        mean_mj(7)
                        mean_mj(6)
                        norms()
                        for mj in range(0, 6):
                            mean_mj(mj)
                        del exps[q]
                        del pvts[q]
                    else:
                        # tail: pair-7 norms first so O-proj starts early;
                        # pair-7 means overlap O-proj
                        norms()
                        mean_mj(7)
                        mean_mj(6)
                        for mj in range(0, 6):
                            mean_mj(mj)
                        del exps[q]
                        del pvts[q]

        # ---------------- phase 3: O-projection + outputs ----------------
        with tc.tile_pool(name="o_ps", bufs=4, space="PSUM") as ops:
            for nj in range(8):
                ost = osp.tile([128, D], f16, tag="ost", name="ost")
                for ob in range(2):
                    ps = ops.tile([128, 512], f32, tag="ps", name="ps")
                    for c in range(8):
                        nc.tensor.matmul(
                            ps[:],
                            outcat[c][:, nj * 128:(nj + 1) * 128],
                            wo_t[c][:, ob * 512:(ob + 1) * 512],
                            start=(c == 0), stop=False)
                    nc.tensor.matmul(
                        ps[:],
                        ones16[:, 0:128],
                        bo_sb[:, ob * 512:(ob + 1) * 512],
                        start=False, stop=True)
                    nc.scalar.copy(ost[:, ob * 512:(ob + 1) * 512], ps[:])
                nc.sync.dma_start(d_out[nj * 128:(nj + 1) * 128, :], ost[:])

            # attn_meanT halves (A|B) in fp16; summed + transposed on host
            for mj in range(8):
                nc.sync.dma_start(d_am16[mj * 128:(mj + 1) * 128, :],
                                  acc2[mj][:])

        osp.release()
        wop.release()
        persist.release()
        const.release()

    nc.compile()
    _CACHE["nc"] = nc
    return nc


def prepare_in_maps(query, key, value, w_q, b_q, w_k, b_k, w_v, b_v, w_o, b_o):
    """Host-side sharding + layout prep. Returns list of per-core input dicts."""
    f = np.float32
    h = np.float16
    wqT = np.ascontiguousarray(np.asarray(w_q, f).T.astype(h))
    wkT = np.ascontiguousarray(np.asarray(w_k, f).T.astype(h))
    wvT = np.ascontiguousarray(np.asarray(w_v, f).T.astype(h))
    woT16 = np.ascontiguousarray(np.asarray(w_o, f).T.astype(h))
    bq_pp = np.ascontiguousarray(np.asarray(b_q, f).reshape(8, 128).T)
    bk_pp = np.ascontiguousarray(np.asarray(b_k, f).reshape(8, 128).T)
    bv_r = np.asarray(b_v, f).reshape(1, D).astype(h)
    bo_r = np.asarray(b_o, f).reshape(1, D).astype(h)
    ones16 = np.ones((1, 512), h)
    query = np.asarray(query, f)
    key = np.asarray(key, f)
    value = np.asarray(value, f)

    in_maps = []
    for b in range(B):
        in_maps.append({
            "qT_in": np.ascontiguousarray(query[b].T.astype(h)),
            "kT_in": np.ascontiguousarray(key[b].T.astype(h)),
            "vT_in": np.ascontiguousarray(value[b].T.astype(h)),
            "wqT": wqT, "wkT": wkT, "wvT": wvT, "woT16": woT16,
            "bq_pp": bq_pp, "bk_pp": bk_pp, "bv_r": bv_r, "bo_r": bo_r,
            "ones16": ones16,
        })
    return in_maps


def unpack(res):
    """res -> (out [B,N,D] fp32, attn_mean [B,N,M] fp32)."""
    out = np.stack([res.results[b]["out16"].astype(np.float32)
                    for b in range(B)])
    ams = []
    for b in range(B):
        am16 = res.results[b]["am16"]
        ams.append((am16[:, 0:N].astype(np.float32)
                    + am16[:, N:2 * N].astype(np.float32)).T * (1.0 / H))
    return out, np.stack(ams)


def run(in_maps, trace=False, **kw):
    _install_ntff_hook()
    from concourse.bass_utils import run_bass_kernel_spmd
    nc = build_module()
    return run_bass_kernel_spmd(nc, in_maps, core_ids=list(range(N_CORES)),
                                trace=trace, **kw)


def kernel(query, key, value, w_q, b_q, w_k, b_k, w_v, b_v, w_o, b_o):
    in_maps = prepare_in_maps(query, key, value, w_q, b_q, w_k, b_k,
                              w_v, b_v, w_o, b_o)
    res = run(in_maps)
    return unpack(res)



# revision 57
# speedup vs baseline: 1.1966x; 1.1966x over previous
"""Trainium2 Bass kernel for nn_CrossAttention (B=8, N=M=1024, D=1024, DK=768, H=16).

Sharding: data-parallel over batch B=8 -> one batch item per NeuronCore.
No collectives; attn.mean(dim=1) is over heads, all heads of a batch item
live on one core.

Per-core layout strategy:
  - Host pre-transposes activations and weights so every matmul has its
    contraction dim on SBUF partitions.
  - Scores are computed transposed: scoresT[m, n]. exp() is applied without
    max-subtraction (scores are bounded ~|2|; softmax value is unchanged).
  - QK^T for a head pair runs row-packed (64-row groups at tile_position
    (0,0)/(64,0)) so the two matmuls share the PE array concurrently.
  - PV uses a ones-augmented stationary [v_h | 1] so psum row 64 yields the
    softmax denominators for free.
  - The head-pair loop is software-pipelined: window p issues QK(p)
    interleaved with PV(p-1) at m-tile granularity to keep the PE stream
    dense; normalization/mean work of pair p-1 drains during window p.
  - attn_meanT = sum_h expT_h * (1/(16*sum_h[n])) accumulated on DVE in fp16;
    the host transposes the returned [m, n] array back to [n, m].
  - out = (outcatT * rbc16) @ (16*w_o.T) + b_o  (1/16 folded into the
    normalizer is compensated by scaling w_o.T by 16 on the host).
"""

import sys
import types

sys.path.insert(0, "/opt/trn_rl_repo")
sys.path.insert(0, "/root/.axon_site")

import numpy as np

N_CORES = 8
B, N, M = 8, 1024, 1024
D = 1024      # Q_DIM
DK = 768      # K_DIM
H = 16        # heads
HD = 64       # head dim
SCALE = HD ** -0.5  # 0.125


def _install_ntff_hook():
    """Make trace=True work under axon (antenv.axon_hooks shim)."""
    if "antenv.axon_hooks" in sys.modules:
        return
    try:
        import antenv
        hooks_mod = types.ModuleType("antenv.axon_hooks")
        _hook = [None]
        hooks_mod.set_axon_ntff_profile_hook = lambda h: _hook.__setitem__(0, h)
        hooks_mod.get_axon_ntff_profile_hook = lambda: _hook[0]
        sys.modules["antenv.axon_hooks"] = hooks_mod
        antenv.axon_hooks = hooks_mod
        from trn_agent_boot.trn_boot import _ntff_profile_via_ctypes
        hooks_mod.set_axon_ntff_profile_hook(
            _ntff_profile_via_ctypes("/opt/axon/libaxon_pjrt.so")
        )
    except Exception:
        pass


_CACHE = {}


def build_module():
    if "nc" in _CACHE:
        return _CACHE["nc"]

    import concourse.tile as tile
    import concourse.mybir as mybir
    from concourse import bacc, library_config

    f32 = mybir.dt.float32
    f32r = mybir.dt.float32r
    f16 = mybir.dt.float16
    AF = mybir.ActivationFunctionType

    nc = bacc.Bacc("TRN2", target_bir_lowering=False, debug=False,
                   num_devices=N_CORES)

    # ---- DRAM tensors (per-core shard) ----
    d_qT = nc.dram_tensor("qT_in", [D, N], f16, kind="ExternalInput").ap()
    d_kT = nc.dram_tensor("kT_in", [DK, M], f16, kind="ExternalInput").ap()
    d_vT = nc.dram_tensor("vT_in", [DK, M], f16, kind="ExternalInput").ap()
    d_wqT = nc.dram_tensor("wqT", [D, D], f16, kind="ExternalInput").ap()
    d_wkT = nc.dram_tensor("wkT", [DK, D], f16, kind="ExternalInput").ap()
    d_wvT = nc.dram_tensor("wvT", [DK, D], f16, kind="ExternalInput").ap()
    d_woT = nc.dram_tensor("woT16", [D, D], f16, kind="ExternalInput").ap()
    d_bq = nc.dram_tensor("bq_pp", [128, 8], f32, kind="ExternalInput").ap()
    d_bk = nc.dram_tensor("bk_pp", [128, 8], f32, kind="ExternalInput").ap()
    d_bv = nc.dram_tensor("bv_r", [1, D], f16, kind="ExternalInput").ap()
    d_bo = nc.dram_tensor("bo_r", [1, D], f16, kind="ExternalInput").ap()
    d_ones16 = nc.dram_tensor("ones16", [1, 512], f16, kind="ExternalInput").ap()
    d_out = nc.dram_tensor("out16", [N, D], f16, kind="ExternalOutput").ap()
    d_am16 = nc.dram_tensor("am16", [M, 2 * N], f16, kind="ExternalOutput").ap()

    with tile.TileContext(nc) as tc:
        nc.gpsimd.load_library(library_config.proxy)

        # ---------------- persistent pools ----------------
        const = tc.alloc_tile_pool(name="const", bufs=1)
        persist = tc.alloc_tile_pool(name="persist", bufs=1)

        ones16 = const.tile([1, 512], f16, tag="ones16", name="ones16")
        nc.sync.dma_start(ones16[:], d_ones16[:, :])
        bq_sb = const.tile([128, 8], f32, tag="bq", name="bq")
        bk_sb = const.tile([128, 8], f32, tag="bk", name="bk")

        qT = [persist.tile([128, N], f16, tag=f"qT{j}", name=f"qT{j}")
              for j in range(8)]
        kT = [persist.tile([128, M], f16, tag=f"kT{j}", name=f"kT{j}")
              for j in range(8)]
        v_sb = [persist.tile([128, H, HD + 1], f16, tag=f"v{j}", name=f"v{j}")
                for j in range(8)]
        outcat = [persist.tile([128, N], f16, tag=f"oc{j}", name=f"oc{j}")
                  for j in range(8)]
        # acc2[mj] = [sum_h normalized expT_A | sum_h ... B] (A cols 0:1024,
        # B cols 1024:2048); halves summed on host.
        acc2 = [persist.tile([128, 2 * N], f16, tag=f"acc{j}", name=f"acc{j}")
                for j in range(8)]

        for j in range(8):
            nc.vector.memset(v_sb[j][:, :, HD:HD + 1], 1.0)

        # ---------------- phase 1: projections ----------------
        wkv = tc.alloc_tile_pool(name="wkv", bufs=1)
        wk_t = [wkv.tile([128, D], f16, tag=f"wk{c}", name=f"wk{c}")
                for c in range(6)]
        wv_t = [wkv.tile([128, D], f16, tag=f"wv{c}", name=f"wv{c}")
                for c in range(6)]
        bv_sb = wkv.tile([1, D], f16, tag="bv", name="bv")

        wqp = tc.alloc_tile_pool(name="wqp", bufs=1)
        wq_t = [wqp.tile([128, D], f16, tag=f"wq{c}", name=f"wq{c}")
                for c in range(8)]

        # --- Q projection: qT[do, n] = wqT-chunks.T @ qT_in ---
        with tc.tile_pool(name="xq", bufs=4) as xp, \
             tc.tile_pool(name="proj_ps", bufs=1, space="PSUM") as pps:
            for nb in range(2):
                pss = [pps.tile([128, 512], f32, tag=f"ps{j}", name=f"ps{j}")
                       for j in range(8)]
                for c in range(8):
                    if nb == 0:
                        nc.sync.dma_start(wq_t[c][:],
                                          d_wqT[c * 128:(c + 1) * 128, :])
                    xt = xp.tile([128, 512], f16, tag="x", name="xt")
                    nc.sync.dma_start(
                        xt[:], d_qT[c * 128:(c + 1) * 128,
                                    nb * 512:(nb + 1) * 512])
                    for j in range(8):
                        nc.tensor.matmul(
                            pss[j][:],
                            wq_t[c][:, j * 128:(j + 1) * 128],
                            xt[:],
                            start=(c == 0), stop=(c == 7))
                if nb == 0:
                    nc.sync.dma_start(bq_sb[:], d_bq[:, :])
                    nc.sync.dma_start(bk_sb[:], d_bk[:, :])
                for j in range(8):
                    nc.scalar.activation(
                        qT[j][:, nb * 512:(nb + 1) * 512], pss[j][:],
                        AF.Identity, bias=bq_sb[:, j:j + 1], scale=1.0)
        wqp.release()


        with tc.tile_pool(name="xkv", bufs=4) as xp, \
             tc.tile_pool(name="proj_ps2", bufs=1, space="PSUM") as pps:
            # --- K projection ---
            for nb in range(2):
                pss = [pps.tile([128, 512], f32, tag=f"ps{j}", name=f"ps{j}")
                       for j in range(8)]
                for c in range(6):
                    if nb == 0:
                        nc.sync.dma_start(wk_t[c][:],
                                          d_wkT[c * 128:(c + 1) * 128, :])
                        nc.sync.dma_start(wv_t[c][:],
                                          d_wvT[c * 128:(c + 1) * 128, :])
                    xt = xp.tile([128, 1024], f16, tag="x", name="xt")
                    nc.sync.dma_start(
                        xt[:, 0:512], d_kT[c * 128:(c + 1) * 128,
                                           nb * 512:(nb + 1) * 512])
                    for j in range(8):
                        nc.tensor.matmul(
                            pss[j][:],
                            wk_t[c][:, j * 128:(j + 1) * 128],
                            xt[:, 0:512],
                            start=(c == 0), stop=(c == 5))
                if nb == 0:
                    nc.sync.dma_start(bv_sb[:], d_bv[:, :])
                for j in range(8):
                    nc.scalar.activation(
                        kT[j][:, nb * 512:(nb + 1) * 512], pss[j][:],
                        AF.Identity, bias=bk_sb[:, j:j + 1], scale=1.0)

            # --- V projection: v[m, do] = vT_in-chunks.T @ wvT (+ b_v) ---
            for ob in range(2):
                pss = [pps.tile([128, 512], f32, tag=f"ps{j}", name=f"ps{j}")
                       for j in range(8)]
                for c in range(6):
                    xt = xp.tile([128, 1024], f16, tag="x", name="xt")
                    nc.sync.dma_start(xt[:], d_vT[c * 128:(c + 1) * 128, :])
                    for mj in range(8):
                        nc.tensor.matmul(
                            pss[mj][:],
                            xt[:, mj * 128:(mj + 1) * 128],
                            wv_t[c][:, ob * 512:(ob + 1) * 512],
                            start=(c == 0), stop=False)
                for mj in range(8):
                    nc.tensor.matmul(
                        pss[mj][:],
                        ones16[:, 0:128],
                        bv_sb[:, ob * 512:(ob + 1) * 512],
                        start=False, stop=True)
                    nc.scalar.activation(
                        v_sb[mj][:, ob * 8:(ob + 1) * 8, 0:HD],
                        pss[mj][:].rearrange("p (a b) -> p a b", a=8),
                        AF.Copy)
        wkv.release()

        # w_o loaded during attention so O-projection starts without a stall
        wop = tc.alloc_tile_pool(name="wo", bufs=1)
        wo_t = [wop.tile([128, D], f16, tag=f"wo{c}", name=f"wo{c}")
                for c in range(8)]
        bo_sb = wop.tile([1, D], f16, tag="bo", name="bo")
        nc.sync.dma_start(bo_sb[:], d_bo[:, :])
        for c in range(8):
            nc.sync.dma_start(wo_t[c][:], d_woT[c * 128:(c + 1) * 128, :])

        # O-proj fp16 staging (bufs=1: copy/DMA alternate in the tail)
        osp = tc.alloc_tile_pool(name="ostage", bufs=1)

        # ---------------- phase 2: attention (software-pipelined pairs) ----
        with tc.tile_pool(name="exp", bufs=2) as expp, \
             tc.tile_pool(name="att_tmp", bufs=2) as tmpp, \
             tc.tile_pool(name="rinvp", bufs=1) as rinvp, \
             tc.tile_pool(name="rbc", bufs=1) as rbcp, \
             tc.tile_pool(name="qk_ps", bufs=1, space="PSUM") as qkps, \
             tc.tile_pool(name="pv_ps", bufs=1, space="PSUM") as pvps:

            exps = {}
            pvts = {}
            deferred = []
            for p in range(9):
                # ---- interleaved PE stream: QK(p) + PV(p-1) ----
                if p < 8:
                    cur = []
                    exps[p] = cur
                for mj in range(8):
                    if p < 8:
                        et = expp.tile([128, 2 * N], f16, tag=f"exp{mj}",
                                       name=f"exp{mj}", bufs=2)
                        psA = qkps.tile([128, 1024], f32, tag="qkA",
                                        name="qkA")
                        for nb in range(2):
                            nc.tensor.matmul(
                                psA[:, nb * 512:(nb + 1) * 512],
                                kT[p][0:64, mj * 128:(mj + 1) * 128],
                                qT[p][0:64, nb * 512:(nb + 1) * 512],
                                start=True, stop=True, tile_position=(0, 0))
                        nc.scalar.activation(et[:, 0:N], psA[:], AF.Exp,
                                             scale=SCALE)
                        psB = qkps.tile([128, 1024], f32, tag="qkB",
                                        name="qkB")
                        for nb in range(2):
                            nc.tensor.matmul(
                                psB[:, nb * 512:(nb + 1) * 512],
                                kT[p][64:128, mj * 128:(mj + 1) * 128],
                                qT[p][64:128, nb * 512:(nb + 1) * 512],
                                start=True, stop=True, tile_position=(64, 0))
                        nc.scalar.activation(et[:, N:2 * N], psB[:], AF.Exp,
                                             scale=SCALE)
                    if p >= 1:
                        pvA, pvB = pvts[p - 1]
                        c = exps[p - 1]
                        for nb in range(2):
                            nc.tensor.matmul(
                                pvA[:, nb * 512:(nb + 1) * 512],
                                v_sb[mj][:, 2 * (p - 1), :],
                                c[mj][:, nb * 512:(nb + 1) * 512],
                                start=(mj == 0), stop=(mj == 7))
                            nc.tensor.matmul(
                                pvB[:, nb * 512:(nb + 1) * 512],
                                v_sb[mj][:, 2 * (p - 1) + 1, :],
                                c[mj][:, N + nb * 512:N + (nb + 1) * 512],
                                start=(mj == 0), stop=(mj == 7))
                    if p < 8:
                        cur.append(et)
                if p < 8:
                    pvts[p] = (
                        pvps.tile([65, 1024], f32, tag="pvA", name="pvA"),
                        pvps.tile([65, 1024], f32, tag="pvB", name="pvB"))

                # ---- drain pair p-1 ----
                if p >= 1:
                    q = p - 1
                    pvA, pvB = pvts[q]
                    c = exps[q]
                    sCAT = rinvp.tile([1, 2048], f32, tag="sCAT",
                                      name="sCAT")
                    r16cat = rinvp.tile([1, 2048], f16, tag="r16cat",
                                        name="r16cat", bufs=1)
                    # den extraction first: it heads the critical chain
                    # den -> recip -> cast -> broadcast -> means
                    nc.scalar.copy(sCAT[:, 0:N], pvA[64:65, :])
                    nc.scalar.copy(sCAT[:, N:2 * N], pvB[64:65, :])
                    nc.vector.reciprocal_approx_fast(out=sCAT[:], in_=sCAT[:])
                    nc.vector.tensor_copy(r16cat[:], sCAT[:])
                    rbcAB = rbcp.tile([128, 2 * N], f16, tag="rbcAB",
                                      name="rbcAB", bufs=1)
                    nc.gpsimd.partition_broadcast(rbcAB[:], r16cat[:])

                    def mean_mj(mj, q=q, c=c, rbcAB=rbcAB):
                        if q == 0:
                            nc.vector.tensor_mul(acc2[mj][:], c[mj][:],
                                                 rbcAB[:])
                        else:
                            at = tmpp.tile([128, 2 * N], f16, tag="at",
                                           name="at", bufs=1)
                            nc.vector.tensor_mul(at[:], c[mj][:], rbcAB[:])
                            nc.vector.tensor_add(acc2[mj][:], acc2[mj][:],
                                                 at[:])

                    def norms(q=q, rbcAB=rbcAB, pvA=pvA, pvB=pvB):
                        # fused psum-read normalize (copy+mul in one TT)
                        nc.vector.tensor_mul(outcat[q][0:64, :],
                                             pvA[0:64, :],
                                             rbcAB[0:64, 0:N])
                        nc.vector.tensor_mul(outcat[q][64:128, :],
                                             pvB[0:64, :],
                                             rbcAB[64:128, N:2 * N])

                    if p < 8:
                        # gpsimd means (7,6) queued right after its bcast;
                        # DVE: first head-norms, then its means
                        mean_mj(7)
                        mean_mj(6)
                        norms()
                        for mj in range(0, 6):
                            mean_mj(mj)
                        del exps[q]
                        del pvts[q]
                    else:
                        # tail: pair-7 norms first so O-proj starts early;
                        # pair-7 means overlap O-proj
                        norms()
                        mean_mj(7)
                        mean_mj(6)
                        for mj in range(0, 6):
                            mean_mj(mj)
                        del exps[q]
                        del pvts[q]

        # ---------------- phase 3: O-projection + outputs ----------------
        with tc.tile_pool(name="o_ps", bufs=4, space="PSUM") as ops:
            for nj in range(8):
                ost = osp.tile([128, D], f16, tag="ost", name="ost")
                for ob in range(2):
                    ps = ops.tile([128, 512], f32, tag="ps", name="ps")
                    for c in range(8):
                        nc.tensor.matmul(
                            ps[:],
                            outcat[c][:, nj * 128:(nj + 1) * 128],
                            wo_t[c][:, ob * 512:(ob + 1) * 512],
                            start=(c == 0), stop=False)
                    nc.tensor.matmul(
                        ps[:],
                        ones16[:, 0:128],
                        bo_sb[:, ob * 512:(ob + 1) * 512],
                        start=False, stop=True)
                    nc.scalar.copy(ost[:, ob * 512:(ob + 1) * 512], ps[:])
                nc.sync.dma_start(d_out[nj * 128:(nj + 1) * 128, :], ost[:])

            # attn_meanT halves (A|B) in fp16; summed + transposed on host
            for mj in range(8):
                nc.sync.dma_start(d_am16[mj * 128:(mj + 1) * 128, :],
                                  acc2[mj][:])

        osp.release()
        wop.release()
        persist.release()
        const.release()

    nc.compile()
    _CACHE["nc"] = nc
    return nc


def prepare_in_maps(query, key, value, w_q, b_q, w_k, b_k, w_v, b_v, w_o, b_o):
    """Host-side sharding + layout prep. Returns list of per-core input dicts."""
    f = np.float32
    h = np.float16
    wqT = np.ascontiguousarray(np.asarray(w_q, f).T.astype(h))
    wkT = np.ascontiguousarray(np.asarray(w_k, f).T.astype(h))
    wvT = np.ascontiguousarray(np.asarray(w_v, f).T.astype(h))
    woT16 = np.ascontiguousarray(np.asarray(w_o, f).T.astype(h))
    bq_pp = np.ascontiguousarray(np.asarray(b_q, f).reshape(8, 128).T)
    bk_pp = np.ascontiguousarray(np.asarray(b_k, f).reshape(8, 128).T)
    bv_r = np.asarray(b_v, f).reshape(1, D).astype(h)
    bo_r = np.asarray(b_o, f).reshape(1, D).astype(h)
    ones16 = np.ones((1, 512), h)
    query = np.asarray(query, f)
    key = np.asarray(key, f)
    value = np.asarray(value, f)

    in_maps = []
    for b in range(B):
        in_maps.append({
            "qT_in": np.ascontiguousarray(query[b].T.astype(h)),
            "kT_in": np.ascontiguousarray(key[b].T.astype(h)),
            "vT_in": np.ascontiguousarray(value[b].T.astype(h)),
            "wqT": wqT, "wkT": wkT, "wvT": wvT, "woT16": woT16,
            "bq_pp": bq_pp, "bk_pp": bk_pp, "bv_r": bv_r, "bo_r": bo_r,
            "ones16": ones16,
        })
    return in_maps


def unpack(res):
    """res -> (out [B,N,D] fp32, attn_mean [B,N,M] fp32)."""
    out = np.stack([res.results[b]["out16"].astype(np.float32)
                    for b in range(B)])
    ams = []
    for b in range(B):
        am16 = res.results[b]["am16"]
        ams.append((am16[:, 0:N].astype(np.float32)
                    + am16[:, N:2 * N].astype(np.float32)).T * (1.0 / H))
    return out, np.stack(ams)


def run(in_maps, trace=False, **kw):
    _install_ntff_hook()
    from concourse.bass_utils import run_bass_kernel_spmd
    nc = build_module()
    return run_bass_kernel_spmd(nc, in_maps, core_ids=list(range(N_CORES)),
                                trace=trace, **kw)


def kernel(query, key, value, w_q, b_q, w_k, b_k, w_v, b_v, w_o, b_o):
    in_maps = prepare_in_maps(query, key, value, w_q, b_q, w_k, b_k,
                              w_v, b_v, w_o, b_o)
    res = run(in_maps)
    return unpack(res)



# revision 59
# speedup vs baseline: 1.2301x; 1.0280x over previous
"""Trainium2 Bass kernel for nn_CrossAttention (B=8, N=M=1024, D=1024, DK=768, H=16).

Sharding: data-parallel over batch B=8 -> one batch item per NeuronCore.
No collectives; attn.mean(dim=1) is over heads, all heads of a batch item
live on one core.

Per-core layout strategy:
  - Host pre-transposes activations and weights so every matmul has its
    contraction dim on SBUF partitions.
  - Scores are computed transposed: scoresT[m, n]. exp() is applied without
    max-subtraction (scores are bounded ~|2|; softmax value is unchanged).
  - QK^T for a head pair runs row-packed (64-row groups at tile_position
    (0,0)/(64,0)) so the two matmuls share the PE array concurrently.
  - PV uses a ones-augmented stationary [v_h | 1] so psum row 64 yields the
    softmax denominators for free.
  - The head-pair loop is software-pipelined: window p issues QK(p)
    interleaved with PV(p-1) at m-tile granularity to keep the PE stream
    dense; normalization/mean work of pair p-1 drains during window p.
  - attn_meanT = sum_h expT_h * (1/(16*sum_h[n])) accumulated on DVE in fp16;
    the host transposes the returned [m, n] array back to [n, m].
  - out = (outcatT * rbc16) @ (16*w_o.T) + b_o  (1/16 folded into the
    normalizer is compensated by scaling w_o.T by 16 on the host).
"""

import sys
import types

sys.path.insert(0, "/opt/trn_rl_repo")
sys.path.insert(0, "/root/.axon_site")

import numpy as np

N_CORES = 8
B, N, M = 8, 1024, 1024
D = 1024      # Q_DIM
DK = 768      # K_DIM
H = 16        # heads
HD = 64       # head dim
SCALE = HD ** -0.5  # 0.125


def _install_ntff_hook():
    """Make trace=True work under axon (antenv.axon_hooks shim)."""
    if "antenv.axon_hooks" in sys.modules:
        return
    try:
        import antenv
        hooks_mod = types.ModuleType("antenv.axon_hooks")
        _hook = [None]
        hooks_mod.set_axon_ntff_profile_hook = lambda h: _hook.__setitem__(0, h)
        hooks_mod.get_axon_ntff_profile_hook = lambda: _hook[0]
        sys.modules["antenv.axon_hooks"] = hooks_mod
        antenv.axon_hooks = hooks_mod
        from trn_agent_boot.trn_boot import _ntff_profile_via_ctypes
        hooks_mod.set_axon_ntff_profile_hook(
            _ntff_profile_via_ctypes("/opt/axon/libaxon_pjrt.so")
        )
    except Exception:
        pass


_CACHE = {}


def build_module():
    if "nc" in _CACHE:
        return _CACHE["nc"]

    import concourse.tile as tile
    import concourse.mybir as mybir
    from concourse import bacc, library_config

    f32 = mybir.dt.float32
    f32r = mybir.dt.float32r
    f16 = mybir.dt.float16
    AF = mybir.ActivationFunctionType

    nc = bacc.Bacc("TRN2", target_bir_lowering=False, debug=False,
                   num_devices=N_CORES)

    # ---- DRAM tensors (per-core shard) ----
    d_qT = nc.dram_tensor("qT_in", [D, N], f16, kind="ExternalInput").ap()
    d_kT = nc.dram_tensor("kT_in", [DK, M], f16, kind="ExternalInput").ap()
    d_vT = nc.dram_tensor("vT_in", [DK, M], f16, kind="ExternalInput").ap()
    d_wqT = nc.dram_tensor("wqT", [D, D], f16, kind="ExternalInput").ap()
    d_wkT = nc.dram_tensor("wkT", [DK, D], f16, kind="ExternalInput").ap()
    d_wvT = nc.dram_tensor("wvT", [DK, D], f16, kind="ExternalInput").ap()
    d_woT = nc.dram_tensor("woT16", [D, D], f16, kind="ExternalInput").ap()
    d_bq = nc.dram_tensor("bq_pp", [128, 8], f32, kind="ExternalInput").ap()
    d_bk = nc.dram_tensor("bk_pp", [128, 8], f32, kind="ExternalInput").ap()
    d_bv = nc.dram_tensor("bv_r", [1, D], f16, kind="ExternalInput").ap()
    d_bo = nc.dram_tensor("bo_r", [1, D], f16, kind="ExternalInput").ap()
    d_ones16 = nc.dram_tensor("ones16", [1, 512], f16, kind="ExternalInput").ap()
    d_out = nc.dram_tensor("out16", [N, D], f16, kind="ExternalOutput").ap()
    d_am16 = nc.dram_tensor("am16", [M, 2 * N], f16, kind="ExternalOutput").ap()

    with tile.TileContext(nc) as tc:
        nc.gpsimd.load_library(library_config.proxy)

        # ---------------- persistent pools ----------------
        const = tc.alloc_tile_pool(name="const", bufs=1)
        persist = tc.alloc_tile_pool(name="persist", bufs=1)

        ones16 = const.tile([1, 512], f16, tag="ones16", name="ones16")
        nc.sync.dma_start(ones16[:], d_ones16[:, :])
        bq_sb = const.tile([128, 8], f32, tag="bq", name="bq")
        bk_sb = const.tile([128, 8], f32, tag="bk", name="bk")

        qT = [persist.tile([128, N], f16, tag=f"qT{j}", name=f"qT{j}")
              for j in range(8)]
        kT = [persist.tile([128, M], f16, tag=f"kT{j}", name=f"kT{j}")
              for j in range(8)]
        v_sb = [persist.tile([128, H, HD + 1], f16, tag=f"v{j}", name=f"v{j}")
                for j in range(8)]
        outcat = [persist.tile([128, N], f16, tag=f"oc{j}", name=f"oc{j}")
                  for j in range(8)]
        # acc2[mj] = [sum_h normalized expT_A | sum_h ... B] (A cols 0:1024,
        # B cols 1024:2048); halves summed on host.
        acc2 = [persist.tile([128, 2 * N], f16, tag=f"acc{j}", name=f"acc{j}")
                for j in range(8)]

        for j in range(8):
            nc.vector.memset(v_sb[j][:, :, HD:HD + 1], 1.0)

        # ---------------- phase 1: projections ----------------
        wkv = tc.alloc_tile_pool(name="wkv", bufs=1)
        wk_t = [wkv.tile([128, D], f16, tag=f"wk{c}", name=f"wk{c}")
                for c in range(6)]
        wv_t = [wkv.tile([128, D], f16, tag=f"wv{c}", name=f"wv{c}")
                for c in range(6)]
        bv_sb = wkv.tile([1, D], f16, tag="bv", name="bv")

        wqp = tc.alloc_tile_pool(name="wqp", bufs=1)
        wq_t = [wqp.tile([128, D], f16, tag=f"wq{c}", name=f"wq{c}")
                for c in range(8)]

        # --- Q projection: qT[do, n] = wqT-chunks.T @ qT_in ---
        with tc.tile_pool(name="xq", bufs=4) as xp, \
             tc.tile_pool(name="proj_ps", bufs=1, space="PSUM") as pps:
            for nb in range(2):
                pss = [pps.tile([128, 512], f32, tag=f"ps{j}", name=f"ps{j}")
                       for j in range(8)]
                for c in range(8):
                    if nb == 0:
                        nc.sync.dma_start(wq_t[c][:],
                                          d_wqT[c * 128:(c + 1) * 128, :])
                    xt = xp.tile([128, 512], f16, tag="x", name="xt")
                    nc.sync.dma_start(
                        xt[:], d_qT[c * 128:(c + 1) * 128,
                                    nb * 512:(nb + 1) * 512])
                    for j in range(8):
                        nc.tensor.matmul(
                            pss[j][:],
                            wq_t[c][:, j * 128:(j + 1) * 128],
                            xt[:],
                            start=(c == 0), stop=(c == 7))
                if nb == 0:
                    nc.sync.dma_start(bq_sb[:], d_bq[:, :])
                    nc.sync.dma_start(bk_sb[:], d_bk[:, :])
                for j in range(8):
                    nc.scalar.activation(
                        qT[j][:, nb * 512:(nb + 1) * 512], pss[j][:],
                        AF.Identity, bias=bq_sb[:, j:j + 1], scale=1.0)
        wqp.release()


        with tc.tile_pool(name="xkv", bufs=4) as xp, \
             tc.tile_pool(name="proj_ps2", bufs=1, space="PSUM") as pps:
            # --- K projection ---
            for nb in range(2):
                pss = [pps.tile([128, 512], f32, tag=f"ps{j}", name=f"ps{j}")
                       for j in range(8)]
                for c in range(6):
                    if nb == 0:
                        nc.sync.dma_start(wk_t[c][:],
                                          d_wkT[c * 128:(c + 1) * 128, :])
                        nc.sync.dma_start(wv_t[c][:],
                                          d_wvT[c * 128:(c + 1) * 128, :])
                    xt = xp.tile([128, 1024], f16, tag="x", name="xt")
                    nc.sync.dma_start(
                        xt[:, 0:512], d_kT[c * 128:(c + 1) * 128,
                                           nb * 512:(nb + 1) * 512])
                    for j in range(8):
                        nc.tensor.matmul(
                            pss[j][:],
                            wk_t[c][:, j * 128:(j + 1) * 128],
                            xt[:, 0:512],
                            start=(c == 0), stop=(c == 5))
                if nb == 0:
                    nc.sync.dma_start(bv_sb[:], d_bv[:, :])
                for j in range(8):
                    nc.scalar.activation(
                        kT[j][:, nb * 512:(nb + 1) * 512], pss[j][:],
                        AF.Identity, bias=bk_sb[:, j:j + 1], scale=1.0)

            # --- V projection: v[m, do] = vT_in-chunks.T @ wvT (+ b_v) ---
            for ob in range(2):
                pss = [pps.tile([128, 512], f32, tag=f"ps{j}", name=f"ps{j}")
                       for j in range(8)]
                for c in range(6):
                    xt = xp.tile([128, 1024], f16, tag="x", name="xt")
                    nc.sync.dma_start(xt[:], d_vT[c * 128:(c + 1) * 128, :])
                    for mj in range(8):
                        nc.tensor.matmul(
                            pss[mj][:],
                            xt[:, mj * 128:(mj + 1) * 128],
                            wv_t[c][:, ob * 512:(ob + 1) * 512],
                            start=(c == 0), stop=False)
                for mj in range(8):
                    nc.tensor.matmul(
                        pss[mj][:],
                        ones16[:, 0:128],
                        bv_sb[:, ob * 512:(ob + 1) * 512],
                        start=False, stop=True)
                    nc.scalar.activation(
                        v_sb[mj][:, ob * 8:(ob + 1) * 8, 0:HD],
                        pss[mj][:].rearrange("p (a b) -> p a b", a=8),
                        AF.Copy)
        wkv.release()

        # w_o loaded during attention so O-projection starts without a stall
        wop = tc.alloc_tile_pool(name="wo", bufs=1)
        wo_t = [wop.tile([128, D], f16, tag=f"wo{c}", name=f"wo{c}")
                for c in range(8)]
        bo_sb = wop.tile([1, D], f16, tag="bo", name="bo")
        nc.sync.dma_start(bo_sb[:], d_bo[:, :])
        for c in range(8):
            nc.sync.dma_start(wo_t[c][:], d_woT[c * 128:(c + 1) * 128, :])

        # O-proj fp16 staging (bufs=1: copy/DMA alternate in the tail)
        osp = tc.alloc_tile_pool(name="ostage", bufs=1)

        # ---------------- phase 2: attention (software-pipelined pairs) ----
        with tc.tile_pool(name="exp", bufs=2) as expp, \
             tc.tile_pool(name="att_tmp", bufs=2) as tmpp, \
             tc.tile_pool(name="rinvp", bufs=1) as rinvp, \
             tc.tile_pool(name="rbc", bufs=1) as rbcp, \
             tc.tile_pool(name="qk_ps", bufs=1, space="PSUM") as qkps, \
             tc.tile_pool(name="pv_ps", bufs=1, space="PSUM") as pvps:

            exps = {}
            pvts = {}
            deferred = []
            for p in range(9):
                # ---- interleaved PE stream: QK(p) + PV(p-1) ----
                if p < 8:
                    cur = []
                    exps[p] = cur
                for mj in range(8):
                    if p < 8:
                        et = expp.tile([128, 2 * N], f16, tag=f"exp{mj}",
                                       name=f"exp{mj}", bufs=2)
                        psA = qkps.tile([128, 1024], f32, tag="qkA",
                                        name="qkA")
                        for nb in range(2):
                            nc.tensor.matmul(
                                psA[:, nb * 512:(nb + 1) * 512],
                                kT[p][0:64, mj * 128:(mj + 1) * 128],
                                qT[p][0:64, nb * 512:(nb + 1) * 512],
                                start=True, stop=True, tile_position=(0, 0))
                        nc.scalar.activation(et[:, 0:N], psA[:], AF.Exp,
                                             scale=SCALE)
                        psB = qkps.tile([128, 1024], f32, tag="qkB",
                                        name="qkB")
                        for nb in range(2):
                            nc.tensor.matmul(
                                psB[:, nb * 512:(nb + 1) * 512],
                                kT[p][64:128, mj * 128:(mj + 1) * 128],
                                qT[p][64:128, nb * 512:(nb + 1) * 512],
                                start=True, stop=True, tile_position=(64, 0))
                        nc.scalar.activation(et[:, N:2 * N], psB[:], AF.Exp,
                                             scale=SCALE)
                    if p >= 1:
                        pvA, pvB = pvts[p - 1]
                        c = exps[p - 1]
                        for nb in range(2):
                            nc.tensor.matmul(
                                pvA[:, nb * 512:(nb + 1) * 512],
                                v_sb[mj][:, 2 * (p - 1), :],
                                c[mj][:, nb * 512:(nb + 1) * 512],
                                start=(mj == 0), stop=(mj == 7))
                            nc.tensor.matmul(
                                pvB[:, nb * 512:(nb + 1) * 512],
                                v_sb[mj][:, 2 * (p - 1) + 1, :],
                                c[mj][:, N + nb * 512:N + (nb + 1) * 512],
                                start=(mj == 0), stop=(mj == 7))
                    if p < 8:
                        cur.append(et)
                if p < 8:
                    pvts[p] = (
                        pvps.tile([65, 1024], f32, tag="pvA", name="pvA"),
                        pvps.tile([65, 1024], f32, tag="pvB", name="pvB"))

                # ---- drain pair p-1 ----
                if p >= 1:
                    q = p - 1
                    pvA, pvB = pvts[q]
                    c = exps[q]
                    sCAT = rinvp.tile([1, 2048], f32, tag="sCAT",
                                      name="sCAT")
                    r16cat = rinvp.tile([1, 2048], f16, tag="r16cat",
                                        name="r16cat", bufs=1)
                    # den extraction first: it heads the critical chain
                    # den -> recip -> cast -> broadcast -> means
                    nc.scalar.copy(sCAT[:, 0:N], pvA[64:65, :])
                    nc.scalar.copy(sCAT[:, N:2 * N], pvB[64:65, :])
                    # outcat copies right after: they free the PV psum for
                    # the next pair's PV accumulation
                    nc.scalar.copy(outcat[q][0:64, :], pvA[0:64, :])
                    nc.scalar.copy(outcat[q][64:128, :], pvB[0:64, :])
                    nc.vector.reciprocal_approx_fast(out=sCAT[:], in_=sCAT[:])
                    nc.vector.tensor_copy(r16cat[:], sCAT[:])
                    rbcAB = rbcp.tile([128, 2 * N], f16, tag="rbcAB",
                                      name="rbcAB", bufs=1)
                    nc.gpsimd.partition_broadcast(rbcAB[:], r16cat[:])

                    def mean_mj(mj, q=q, c=c, rbcAB=rbcAB):
                        if q == 0:
                            nc.vector.tensor_mul(acc2[mj][:], c[mj][:],
                                                 rbcAB[:])
                        else:
                            at = tmpp.tile([128, 2 * N], f16, tag="at",
                                           name="at", bufs=1)
                            nc.vector.tensor_mul(at[:], c[mj][:], rbcAB[:])
                            nc.vector.tensor_add(acc2[mj][:], acc2[mj][:],
                                                 at[:])

                    def norms(q=q, rbcAB=rbcAB):
                        nc.vector.tensor_mul(outcat[q][0:64, :],
                                             outcat[q][0:64, :],
                                             rbcAB[0:64, 0:N])
                        nc.vector.tensor_mul(outcat[q][64:128, :],
                                             outcat[q][64:128, :],
                                             rbcAB[64:128, N:2 * N])

                    if p < 8:
                        # gpsimd means (7,6) queued right after its bcast;
                        # DVE: first head-norms, then its means
                        mean_mj(7)
                        mean_mj(6)
                        norms()
                        for mj in range(0, 6):
                            mean_mj(mj)
                        del exps[q]
                        del pvts[q]
                    else:
                        # tail: pair-7 norms first so O-proj starts early;
                        # pair-7 means overlap O-proj
                        norms()
                        mean_mj(7)
                        mean_mj(6)
                        for mj in range(0, 6):
                            mean_mj(mj)
                        del exps[q]
                        del pvts[q]

        # ---------------- phase 3: O-projection + outputs ----------------
        with tc.tile_pool(name="o_ps", bufs=4, space="PSUM") as ops:
            for nj in range(8):
                ost = osp.tile([128, D], f16, tag="ost", name="ost")
                for ob in range(2):
                    ps = ops.tile([128, 512], f32, tag="ps", name="ps")
                    for c in range(8):
                        nc.tensor.matmul(
                            ps[:],
                            outcat[c][:, nj * 128:(nj + 1) * 128],
                            wo_t[c][:, ob * 512:(ob + 1) * 512],
                            start=(c == 0), stop=False)
                    nc.tensor.matmul(
                        ps[:],
                        ones16[:, 0:128],
                        bo_sb[:, ob * 512:(ob + 1) * 512],
                        start=False, stop=True)
                    nc.scalar.copy(ost[:, ob * 512:(ob + 1) * 512], ps[:])
                nc.sync.dma_start(d_out[nj * 128:(nj + 1) * 128, :], ost[:])

            # attn_meanT halves (A|B) in fp16; summed + transposed on host
            for mj in range(8):
                nc.sync.dma_start(d_am16[mj * 128:(mj + 1) * 128, :],
                                  acc2[mj][:])

        osp.release()
        wop.release()
        persist.release()
        const.release()

    nc.compile()
    _CACHE["nc"] = nc
    return nc


def prepare_in_maps(query, key, value, w_q, b_q, w_k, b_k, w_v, b_v, w_o, b_o):
    """Host-side sharding + layout prep. Returns list of per-core input dicts."""
    f = np.float32
    h = np.float16
    wqT = np.ascontiguousarray(np.asarray(w_q, f).T.astype(h))
    wkT = np.ascontiguousarray(np.asarray(w_k, f).T.astype(h))
    wvT = np.ascontiguousarray(np.asarray(w_v, f).T.astype(h))
    woT16 = np.ascontiguousarray(np.asarray(w_o, f).T.astype(h))
    bq_pp = np.ascontiguousarray(np.asarray(b_q, f).reshape(8, 128).T)
    bk_pp = np.ascontiguousarray(np.asarray(b_k, f).reshape(8, 128).T)
    bv_r = np.asarray(b_v, f).reshape(1, D).astype(h)
    bo_r = np.asarray(b_o, f).reshape(1, D).astype(h)
    ones16 = np.ones((1, 512), h)
    query = np.asarray(query, f)
    key = np.asarray(key, f)
    value = np.asarray(value, f)

    in_maps = []
    for b in range(B):
        in_maps.append({
            "qT_in": np.ascontiguousarray(query[b].T.astype(h)),
            "kT_in": np.ascontiguousarray(key[b].T.astype(h)),
            "vT_in": np.ascontiguousarray(value[b].T.astype(h)),
            "wqT": wqT, "wkT": wkT, "wvT": wvT, "woT16": woT16,
            "bq_pp": bq_pp, "bk_pp": bk_pp, "bv_r": bv_r, "bo_r": bo_r,
            "ones16": ones16,
        })
    return in_maps


def unpack(res):
    """res -> (out [B,N,D] fp32, attn_mean [B,N,M] fp32)."""
    out = np.stack([res.results[b]["out16"].astype(np.float32)
                    for b in range(B)])
    ams = []
    for b in range(B):
        am16 = res.results[b]["am16"]
        ams.append((am16[:, 0:N].astype(np.float32)
                    + am16[:, N:2 * N].astype(np.float32)).T * (1.0 / H))
    return out, np.stack(ams)


def run(in_maps, trace=False, **kw):
    _install_ntff_hook()
    from concourse.bass_utils import run_bass_kernel_spmd
    nc = build_module()
    return run_bass_kernel_spmd(nc, in_maps, core_ids=list(range(N_CORES)),
                                trace=trace, **kw)


def kernel(query, key, value, w_q, b_q, w_k, b_k, w_v, b_v, w_o, b_o):
    in_maps = prepare_in_maps(query, key, value, w_q, b_q, w_k, b_k,
                              w_v, b_v, w_o, b_o)
    res = run(in_maps)
    return unpack(res)



# revision 61
# speedup vs baseline: 1.3637x; 1.1086x over previous
"""Trainium2 Bass kernel for nn_CrossAttention (B=8, N=M=1024, D=1024, DK=768, H=16).

Sharding: data-parallel over batch B=8 -> one batch item per NeuronCore.
No collectives; attn.mean(dim=1) is over heads, all heads of a batch item
live on one core.

Per-core layout strategy:
  - Host pre-transposes activations and weights so every matmul has its
    contraction dim on SBUF partitions.
  - Scores are computed transposed: scoresT[m, n]. exp() is applied without
    max-subtraction (scores are bounded ~|2|; softmax value is unchanged).
  - QK^T for a head pair runs row-packed (64-row groups at tile_position
    (0,0)/(64,0)) so the two matmuls share the PE array concurrently.
  - PV uses a ones-augmented stationary [v_h | 1] so psum row 64 yields the
    softmax denominators for free.
  - The head-pair loop is software-pipelined: window p issues QK(p)
    interleaved with PV(p-1) at m-tile granularity to keep the PE stream
    dense; normalization/mean work of pair p-1 drains during window p.
  - attn_meanT = sum_h expT_h * (1/(16*sum_h[n])) accumulated on DVE in fp16;
    the host transposes the returned [m, n] array back to [n, m].
  - out = (outcatT * rbc16) @ (16*w_o.T) + b_o  (1/16 folded into the
    normalizer is compensated by scaling w_o.T by 16 on the host).
"""

import sys
import types

sys.path.insert(0, "/opt/trn_rl_repo")
sys.path.insert(0, "/root/.axon_site")

import numpy as np

N_CORES = 8
B, N, M = 8, 1024, 1024
D = 1024      # Q_DIM
DK = 768      # K_DIM
H = 16        # heads
HD = 64       # head dim
SCALE = HD ** -0.5  # 0.125


def _install_ntff_hook():
    """Make trace=True work under axon (antenv.axon_hooks shim)."""
    if "antenv.axon_hooks" in sys.modules:
        return
    try:
        import antenv
        hooks_mod = types.ModuleType("antenv.axon_hooks")
        _hook = [None]
        hooks_mod.set_axon_ntff_profile_hook = lambda h: _hook.__setitem__(0, h)
        hooks_mod.get_axon_ntff_profile_hook = lambda: _hook[0]
        sys.modules["antenv.axon_hooks"] = hooks_mod
        antenv.axon_hooks = hooks_mod
        from trn_agent_boot.trn_boot import _ntff_profile_via_ctypes
        hooks_mod.set_axon_ntff_profile_hook(
            _ntff_profile_via_ctypes("/opt/axon/libaxon_pjrt.so")
        )
    except Exception:
        pass


_CACHE = {}


def build_module():
    if "nc" in _CACHE:
        return _CACHE["nc"]

    import concourse.tile as tile
    import concourse.mybir as mybir
    from concourse import bacc, library_config

    f32 = mybir.dt.float32
    f32r = mybir.dt.float32r
    f16 = mybir.dt.float16
    AF = mybir.ActivationFunctionType

    nc = bacc.Bacc("TRN2", target_bir_lowering=False, debug=False,
                   num_devices=N_CORES)

    # ---- DRAM tensors (per-core shard) ----
    d_qT = nc.dram_tensor("qT_in", [D, N], f16, kind="ExternalInput").ap()
    d_kT = nc.dram_tensor("kT_in", [DK, M], f16, kind="ExternalInput").ap()
    d_vT = nc.dram_tensor("vT_in", [DK, M], f16, kind="ExternalInput").ap()
    d_wqT = nc.dram_tensor("wqT", [D, D], f16, kind="ExternalInput").ap()
    d_wkT = nc.dram_tensor("wkT", [DK, D], f16, kind="ExternalInput").ap()
    d_wvT = nc.dram_tensor("wvT", [DK, D], f16, kind="ExternalInput").ap()
    d_woT = nc.dram_tensor("woT16", [D, D], f16, kind="ExternalInput").ap()
    d_bq = nc.dram_tensor("bq_pp", [128, 8], f32, kind="ExternalInput").ap()
    d_bk = nc.dram_tensor("bk_pp", [128, 8], f32, kind="ExternalInput").ap()
    d_bv = nc.dram_tensor("bv_r", [1, D], f16, kind="ExternalInput").ap()
    d_bo = nc.dram_tensor("bo_r", [1, D], f16, kind="ExternalInput").ap()
    d_ones16 = nc.dram_tensor("ones16", [1, 512], f16, kind="ExternalInput").ap()
    d_out = nc.dram_tensor("out16", [N, D], f16, kind="ExternalOutput").ap()
    d_am16 = nc.dram_tensor("am16", [M, 2 * N], f16, kind="ExternalOutput").ap()

    with tile.TileContext(nc) as tc:
        nc.gpsimd.load_library(library_config.proxy)

        # ---------------- persistent pools ----------------
        const = tc.alloc_tile_pool(name="const", bufs=1)
        persist = tc.alloc_tile_pool(name="persist", bufs=1)

        ones16 = const.tile([1, 512], f16, tag="ones16", name="ones16")
        nc.sync.dma_start(ones16[:], d_ones16[:, :])
        bq_sb = const.tile([128, 8], f32, tag="bq", name="bq")
        bk_sb = const.tile([128, 8], f32, tag="bk", name="bk")

        qT = [persist.tile([128, N], f16, tag=f"qT{j}", name=f"qT{j}")
              for j in range(8)]
        kT = [persist.tile([128, M], f16, tag=f"kT{j}", name=f"kT{j}")
              for j in range(8)]
        v_sb = [persist.tile([128, H, HD + 1], f16, tag=f"v{j}", name=f"v{j}")
                for j in range(8)]
        outcat = [persist.tile([128, N], f16, tag=f"oc{j}", name=f"oc{j}")
                  for j in range(8)]
        # acc2[mj] = [sum_h normalized expT_A | sum_h ... B] (A cols 0:1024,
        # B cols 1024:2048); halves summed on host.
        acc2 = [persist.tile([128, 2 * N], f16, tag=f"acc{j}", name=f"acc{j}")
                for j in range(8)]

        for j in range(8):
            nc.vector.memset(v_sb[j][:, :, HD:HD + 1], 1.0)

        # ---------------- phase 1: projections ----------------
        wkv = tc.alloc_tile_pool(name="wkv", bufs=1)
        wk_t = [wkv.tile([128, D], f16, tag=f"wk{c}", name=f"wk{c}")
                for c in range(6)]
        wv_t = [wkv.tile([128, D], f16, tag=f"wv{c}", name=f"wv{c}")
                for c in range(6)]
        bv_sb = wkv.tile([1, D], f16, tag="bv", name="bv")

        wqp = tc.alloc_tile_pool(name="wqp", bufs=1)
        wq_t = [wqp.tile([128, D], f16, tag=f"wq{c}", name=f"wq{c}")
                for c in range(8)]

        # --- Q projection: qT[do, n] = wqT-chunks.T @ qT_in ---
        with tc.tile_pool(name="xq", bufs=4) as xp, \
             tc.tile_pool(name="proj_ps", bufs=1, space="PSUM") as pps:
            for nb in range(2):
                pss = [pps.tile([128, 512], f32, tag=f"ps{j}", name=f"ps{j}")
                       for j in range(8)]
                for c in range(8):
                    if nb == 0:
                        nc.sync.dma_start(wq_t[c][:],
                                          d_wqT[c * 128:(c + 1) * 128, :])
                    xt = xp.tile([128, 512], f16, tag="x", name="xt")
                    nc.sync.dma_start(
                        xt[:], d_qT[c * 128:(c + 1) * 128,
                                    nb * 512:(nb + 1) * 512])
                    for j in range(8):
                        nc.tensor.matmul(
                            pss[j][:],
                            wq_t[c][:, j * 128:(j + 1) * 128],
                            xt[:],
                            start=(c == 0), stop=(c == 7))
                if nb == 0:
                    nc.sync.dma_start(bq_sb[:], d_bq[:, :])
                    nc.sync.dma_start(bk_sb[:], d_bk[:, :])
                for j in range(8):
                    nc.scalar.activation(
                        qT[j][:, nb * 512:(nb + 1) * 512], pss[j][:],
                        AF.Identity, bias=bq_sb[:, j:j + 1], scale=1.0)
        wqp.release()


        with tc.tile_pool(name="xkv", bufs=4) as xp, \
             tc.tile_pool(name="proj_ps2", bufs=1, space="PSUM") as pps:
            # --- K projection ---
            for nb in range(2):
                pss = [pps.tile([128, 512], f32, tag=f"ps{j}", name=f"ps{j}")
                       for j in range(8)]
                for c in range(6):
                    if nb == 0:
                        nc.sync.dma_start(wk_t[c][:],
                                          d_wkT[c * 128:(c + 1) * 128, :])
                        nc.sync.dma_start(wv_t[c][:],
                                          d_wvT[c * 128:(c + 1) * 128, :])
                    xt = xp.tile([128, 1024], f16, tag="x", name="xt")
                    nc.sync.dma_start(
                        xt[:, 0:512], d_kT[c * 128:(c + 1) * 128,
                                           nb * 512:(nb + 1) * 512])
                    for j in range(8):
                        nc.tensor.matmul(
                            pss[j][:],
                            wk_t[c][:, j * 128:(j + 1) * 128],
                            xt[:, 0:512],
                            start=(c == 0), stop=(c == 5))
                if nb == 0:
                    nc.sync.dma_start(bv_sb[:], d_bv[:, :])
                for j in range(8):
                    nc.scalar.activation(
                        kT[j][:, nb * 512:(nb + 1) * 512], pss[j][:],
                        AF.Identity, bias=bk_sb[:, j:j + 1], scale=1.0)

            # --- V projection: v[m, do] = vT_in-chunks.T @ wvT (+ b_v) ---
            for ob in range(2):
                pss = [pps.tile([128, 512], f32, tag=f"ps{j}", name=f"ps{j}")
                       for j in range(8)]
                for c in range(6):
                    xt = xp.tile([128, 1024], f16, tag="x", name="xt")
                    nc.sync.dma_start(xt[:], d_vT[c * 128:(c + 1) * 128, :])
                    for mj in range(8):
                        nc.tensor.matmul(
                            pss[mj][:],
                            xt[:, mj * 128:(mj + 1) * 128],
                            wv_t[c][:, ob * 512:(ob + 1) * 512],
                            start=(c == 0), stop=False)
                for mj in range(8):
                    nc.tensor.matmul(
                        pss[mj][:],
                        ones16[:, 0:128],
                        bv_sb[:, ob * 512:(ob + 1) * 512],
                        start=False, stop=True)
                    nc.scalar.activation(
                        v_sb[mj][:, ob * 8:(ob + 1) * 8, 0:HD],
                        pss[mj][:].rearrange("p (a b) -> p a b", a=8),
                        AF.Copy)
        wkv.release()

        # w_o loaded during attention so O-projection starts without a stall
        wop = tc.alloc_tile_pool(name="wo", bufs=1)
        wo_t = [wop.tile([128, D], f16, tag=f"wo{c}", name=f"wo{c}")
                for c in range(8)]
        bo_sb = wop.tile([1, D], f16, tag="bo", name="bo")
        nc.sync.dma_start(bo_sb[:], d_bo[:, :])
        for c in range(8):
            nc.sync.dma_start(wo_t[c][:], d_woT[c * 128:(c + 1) * 128, :])

        # O-proj fp16 staging (bufs=1: copy/DMA alternate in the tail)
        osp = tc.alloc_tile_pool(name="ostage", bufs=1)

        # ---------------- phase 2: attention (software-pipelined pairs) ----
        with tc.tile_pool(name="exp", bufs=2) as expp, \
             tc.tile_pool(name="att_tmp", bufs=2) as tmpp, \
             tc.tile_pool(name="rinvp", bufs=1) as rinvp, \
             tc.tile_pool(name="rbc", bufs=1) as rbcp, \
             tc.tile_pool(name="qk_ps", bufs=1, space="PSUM") as qkps, \
             tc.tile_pool(name="pv_ps", bufs=1, space="PSUM") as pvps:

            exps = {}
            for p in range(8):
                # ---- window p: QK(p) with PV(p) trailing one mj slot ----
                cur = []
                exps[p] = cur
                pvA = pvps.tile([65, 1024], f32, tag="pvA", name="pvA")
                pvB = pvps.tile([65, 1024], f32, tag="pvB", name="pvB")

                def pv_mj(mj, p=p, pvA=pvA, pvB=pvB):
                    c = exps[p]
                    for nb in range(2):
                        nc.tensor.matmul(
                            pvA[:, nb * 512:(nb + 1) * 512],
                            v_sb[mj][:, 2 * p, :],
                            c[mj][:, nb * 512:(nb + 1) * 512],
                            start=(mj == 0), stop=(mj == 7))
                        nc.tensor.matmul(
                            pvB[:, nb * 512:(nb + 1) * 512],
                            v_sb[mj][:, 2 * p + 1, :],
                            c[mj][:, N + nb * 512:N + (nb + 1) * 512],
                            start=(mj == 0), stop=(mj == 7))

                for mj in range(8):
                    et = expp.tile([128, 2 * N], f16, tag=f"exp{mj}",
                                   name=f"exp{mj}", bufs=2)
                    psA = qkps.tile([128, 1024], f32, tag="qkA",
                                    name="qkA")
                    for nb in range(2):
                        nc.tensor.matmul(
                            psA[:, nb * 512:(nb + 1) * 512],
                            kT[p][0:64, mj * 128:(mj + 1) * 128],
                            qT[p][0:64, nb * 512:(nb + 1) * 512],
                            start=True, stop=True, tile_position=(0, 0))
                    nc.scalar.activation(et[:, 0:N], psA[:], AF.Exp,
                                         scale=SCALE)
                    psB = qkps.tile([128, 1024], f32, tag="qkB",
                                    name="qkB")
                    for nb in range(2):
                        nc.tensor.matmul(
                            psB[:, nb * 512:(nb + 1) * 512],
                            kT[p][64:128, mj * 128:(mj + 1) * 128],
                            qT[p][64:128, nb * 512:(nb + 1) * 512],
                            start=True, stop=True, tile_position=(64, 0))
                    nc.scalar.activation(et[:, N:2 * N], psB[:], AF.Exp,
                                         scale=SCALE)
                    cur.append(et)
                    if mj >= 1:
                        pv_mj(mj - 1)
                pv_mj(7)

                # ---- drain pair p (means run during window p+1) ----
                if True:
                    q = p
                    c = exps[q]
                    sCAT = rinvp.tile([1, 2048], f32, tag="sCAT",
                                      name="sCAT")
                    r16cat = rinvp.tile([1, 2048], f16, tag="r16cat",
                                        name="r16cat", bufs=1)
                    # den extraction first: it heads the critical chain
                    # den -> recip -> cast -> broadcast -> means
                    nc.scalar.copy(sCAT[:, 0:N], pvA[64:65, :])
                    nc.scalar.copy(sCAT[:, N:2 * N], pvB[64:65, :])
                    # outcat copies right after: they free the PV psum for
                    # the next pair's PV accumulation
                    nc.scalar.copy(outcat[q][0:64, :], pvA[0:64, :])
                    nc.scalar.copy(outcat[q][64:128, :], pvB[0:64, :])
                    nc.vector.reciprocal_approx_fast(out=sCAT[:], in_=sCAT[:])
                    nc.vector.tensor_copy(r16cat[:], sCAT[:])
                    rbcAB = rbcp.tile([128, 2 * N], f16, tag="rbcAB",
                                      name="rbcAB", bufs=1)
                    nc.gpsimd.partition_broadcast(rbcAB[:], r16cat[:])

                    def mean_mj(mj, q=q, c=c, rbcAB=rbcAB):
                        if q == 0:
                            nc.vector.tensor_mul(acc2[mj][:], c[mj][:],
                                                 rbcAB[:])
                        else:
                            at = tmpp.tile([128, 2 * N], f16, tag="at",
                                           name="at", bufs=1)
                            nc.vector.tensor_mul(at[:], c[mj][:], rbcAB[:])
                            nc.vector.tensor_add(acc2[mj][:], acc2[mj][:],
                                                 at[:])

                    def norms(q=q, rbcAB=rbcAB):
                        nc.vector.tensor_mul(outcat[q][0:64, :],
                                             outcat[q][0:64, :],
                                             rbcAB[0:64, 0:N])
                        nc.vector.tensor_mul(outcat[q][64:128, :],
                                             outcat[q][64:128, :],
                                             rbcAB[64:128, N:2 * N])

                    if p < 7:
                        # mj=7,6 first: frees et7/et6 buffers earliest
                        mean_mj(7)
                        mean_mj(6)
                        norms()
                        for mj in range(0, 6):
                            mean_mj(mj)
                        del exps[q]
                    else:
                        # last pair: norms first so O-proj starts early;
                        # means overlap O-proj
                        norms()
                        mean_mj(7)
                        mean_mj(6)
                        for mj in range(0, 6):
                            mean_mj(mj)
                        del exps[q]

        # ---------------- phase 3: O-projection + outputs ----------------
        with tc.tile_pool(name="o_ps", bufs=4, space="PSUM") as ops:
            for nj in range(8):
                ost = osp.tile([128, D], f16, tag="ost", name="ost")
                for ob in range(2):
                    ps = ops.tile([128, 512], f32, tag="ps", name="ps")
                    for c in range(8):
                        nc.tensor.matmul(
                            ps[:],
                            outcat[c][:, nj * 128:(nj + 1) * 128],
                            wo_t[c][:, ob * 512:(ob + 1) * 512],
                            start=(c == 0), stop=False)
                    nc.tensor.matmul(
                        ps[:],
                        ones16[:, 0:128],
                        bo_sb[:, ob * 512:(ob + 1) * 512],
                        start=False, stop=True)
                    nc.scalar.copy(ost[:, ob * 512:(ob + 1) * 512], ps[:])
                nc.sync.dma_start(d_out[nj * 128:(nj + 1) * 128, :], ost[:])

            # attn_meanT halves (A|B) in fp16; summed + transposed on host
            for mj in range(8):
                nc.sync.dma_start(d_am16[mj * 128:(mj + 1) * 128, :],
                                  acc2[mj][:])

        osp.release()
        wop.release()
        persist.release()
        const.release()

    nc.compile()
    _CACHE["nc"] = nc
    return nc


def prepare_in_maps(query, key, value, w_q, b_q, w_k, b_k, w_v, b_v, w_o, b_o):
    """Host-side sharding + layout prep. Returns list of per-core input dicts."""
    f = np.float32
    h = np.float16
    wqT = np.ascontiguousarray(np.asarray(w_q, f).T.astype(h))
    wkT = np.ascontiguousarray(np.asarray(w_k, f).T.astype(h))
    wvT = np.ascontiguousarray(np.asarray(w_v, f).T.astype(h))
    woT16 = np.ascontiguousarray(np.asarray(w_o, f).T.astype(h))
    bq_pp = np.ascontiguousarray(np.asarray(b_q, f).reshape(8, 128).T)
    bk_pp = np.ascontiguousarray(np.asarray(b_k, f).reshape(8, 128).T)
    bv_r = np.asarray(b_v, f).reshape(1, D).astype(h)
    bo_r = np.asarray(b_o, f).reshape(1, D).astype(h)
    ones16 = np.ones((1, 512), h)
    query = np.asarray(query, f)
    key = np.asarray(key, f)
    value = np.asarray(value, f)

    in_maps = []
    for b in range(B):
        in_maps.append({
            "qT_in": np.ascontiguousarray(query[b].T.astype(h)),
            "kT_in": np.ascontiguousarray(key[b].T.astype(h)),
            "vT_in": np.ascontiguousarray(value[b].T.astype(h)),
            "wqT": wqT, "wkT": wkT, "wvT": wvT, "woT16": woT16,
            "bq_pp": bq_pp, "bk_pp": bk_pp, "bv_r": bv_r, "bo_r": bo_r,
            "ones16": ones16,
        })
    return in_maps


def unpack(res):
    """res -> (out [B,N,D] fp32, attn_mean [B,N,M] fp32)."""
    out = np.stack([res.results[b]["out16"].astype(np.float32)
                    for b in range(B)])
    ams = []
    for b in range(B):
        am16 = res.results[b]["am16"]
        ams.append((am16[:, 0:N].astype(np.float32)
                    + am16[:, N:2 * N].astype(np.float32)).T * (1.0 / H))
    return out, np.stack(ams)


def run(in_maps, trace=False, **kw):
    _install_ntff_hook()
    from concourse.bass_utils import run_bass_kernel_spmd
    nc = build_module()
    return run_bass_kernel_spmd(nc, in_maps, core_ids=list(range(N_CORES)),
                                trace=trace, **kw)


def kernel(query, key, value, w_q, b_q, w_k, b_k, w_v, b_v, w_o, b_o):
    in_maps = prepare_in_maps(query, key, value, w_q, b_q, w_k, b_k,
                              w_v, b_v, w_o, b_o)
    res = run(in_maps)
    return unpack(res)

